# revision 1
# baseline (speedup 1.0000x reference)
"""Trainium2 Bass kernel for GCNCriticNet (gnn_message_passing).

Structure exploited: the graphs are 8192 independent complete graphs of 16
nodes (plus GCN self-loops), so every node has degree exactly 16, the
symmetric norm is uniformly 1/16, and the GCN aggregation collapses to a
per-graph mean:  agg_i = (1/16) * sum_{j in graph(i)} h_j.
The edge lists therefore never need to be shipped to the device.

Math per layer l (residual, tanh):
  x_l = tanh( x_{l-1} + (sum_graph x_{l-1}) @ (W_l/16) + b_l )
Head: out_g = (sum_graph x_2) @ w_fc1 / 16 + b_fc1

Device layout: activations transposed [HID=128 partitions, node free].
Layer-1 aggregation is fully fused to the input: with
W01 = w_emb @ w_gcn[0]/16, h1 = W01.T @ (graph-sums of obs), and all bias
terms fold into per-partition tanh biases (host-precomputed).

Per 512-node chunk (32 graphs), software-pipelined (stage A of chunk c+1 is
emitted before stage B of chunk c so engine streams interleave):
  A: DMA obs [128, 256] (node = p + 128j); 2 TensorE transposes of
     [128,128] -> psumT [128, 256] (node blocks interleaved [b0 b2 | b1 b3]);
     one [128,256] copy -> obsT; 4 sobs matmuls (graph-sums of obs);
     sobs copy; h1 = W01.T @ sobs; x0 = w_emb.T @ obsT (2 matmuls, PSUM).
  B: u1 = x0 + bcast(h1) (DVE); x1 = tanh(u1 + b1full) (ACT);
     sx1 = group16 reduce (DVE); h2 = (W2/16).T @ sx1; h2 -> SBUF (ACT);
     u2 = x1 + bcast(h2) (Pool/GPSIMD); x2 = tanh(u2 + b2) (ACT);
     sx2 = group16 reduce (DVE); y = w_fc1.T @ sx2 -> out column block (DVE).
Host: undo the block permutation, out = y/16 + b_fc1.
"""

import sys
import numpy as np

try:
    import concourse.bass as bass  # noqa: F401
except ImportError:  # harness runs in a bare dir; repo is on the box
    for p in ("/opt/trn_rl_repo", "/root/.axon_site/_ro/trn_rl_repo"):
        if p not in sys.path:
            sys.path.insert(0, p)
    import concourse.bass as bass  # noqa: F401

import concourse.bacc as bacc
import concourse.mybir as mybir
import concourse.tile as tile
from concourse.bass import MemorySpace
from concourse.bass_utils import run_bass_kernel_spmd

F32 = mybir.dt.float32
AF = mybir.ActivationFunctionType
AX = mybir.AxisListType

N_CORES = 8
N_AGENTS = 16
BATCH = 8192
OBS = 64
HID = 128
N = BATCH * N_AGENTS            # 131072 nodes
NPC = N // N_CORES              # 16384 nodes / core
CHUNK = 1024                    # nodes per inner iteration
NCHUNK = NPC // CHUNK           # 16
GPC = CHUNK // N_AGENTS         # 64 graphs per chunk
OUTPC = NPC // N_AGENTS         # 1024 graphs per core

# node-block order within a chunk after the paired transposes
BLOCK_PERM = (0, 2, 4, 6, 1, 3, 5, 7)
# sobs matmul for block j must write the graph-column slot of block j
BLOCK_POS = {j: i for i, j in enumerate(BLOCK_PERM)}

_CACHE = {}


def _build_nc():
    nc = bacc.Bacc("TRN2", target_bir_lowering=False, debug=False)

    obs_d = nc.dram_tensor("obs", [NPC, OBS], F32, kind="ExternalInput")
    # wemb zero-padded two ways: [0]=[We;0], [1]=[0;We] so both 256-col
    # embedding matmuls contract K=128 from base partition 0
    wemb_d = nc.dram_tensor("wemb", [2, 128, HID], F32, kind="ExternalInput")
    w01_d = nc.dram_tensor("w01", [OBS, HID], F32, kind="ExternalInput")
    w2_d = nc.dram_tensor("w2s", [HID, HID], F32, kind="ExternalInput")
    b1f_d = nc.dram_tensor("b1f", [HID, 1], F32, kind="ExternalInput")
    b2_d = nc.dram_tensor("b2", [HID, 1], F32, kind="ExternalInput")
    wfc_d = nc.dram_tensor("wfc", [HID, 1], F32, kind="ExternalInput")
    p8_d = nc.dram_tensor("p8", [128, 8], F32, kind="ExternalInput")
    id_d = nc.dram_tensor("ident", [128, 128], F32, kind="ExternalInput")
    out_d = nc.dram_tensor("out", [1, OUTPC], F32, kind="ExternalOutput")

    obs_v = obs_d[:].rearrange("(c j p) o -> c p j o", j=8, p=128)

    with tile.TileContext(nc) as tc:
        with (
            tc.tile_pool(name="const", bufs=1) as cp,
            tc.tile_pool(name="io", bufs=4) as iop,
            tc.tile_pool(name="work", bufs=3) as wp,
            tc.tile_pool(name="pT", bufs=1, space=MemorySpace.PSUM) as pTp,
            tc.tile_pool(name="px0", bufs=2, space=MemorySpace.PSUM) as px0p,
            tc.tile_pool(name="psm", bufs=3, space=MemorySpace.PSUM) as psmp,
        ):
            wembA = cp.tile([128, HID], F32)
            nc.sync.dma_start(wembA[:], wemb_d[0])
            wembB = cp.tile([128, HID], F32)
            nc.sync.dma_start(wembB[:], wemb_d[1])
            w01 = cp.tile([OBS, HID], F32)
            nc.sync.dma_start(w01[:], w01_d[:])
            w2 = cp.tile([HID, HID], F32)
            nc.sync.dma_start(w2[:], w2_d[:])
            b1f = cp.tile([HID, 1], F32)
            nc.sync.dma_start(b1f[:], b1f_d[:])
            b2 = cp.tile([HID, 1], F32)
            nc.sync.dma_start(b2[:], b2_d[:])
            wfc = cp.tile([HID, 1], F32)
            nc.sync.dma_start(wfc[:], wfc_d[:])
            p8 = cp.tile([128, 8], F32)
            nc.sync.dma_start(p8[:], p8_d[:])
            ident = cp.tile([128, 128], F32)
            nc.sync.dma_start(ident[:], id_d[:])
            outsb = cp.tile([1, OUTPC], F32)

            stash = {}

            def stage_a(c):
                obs_t = iop.tile([128, 8 * OBS], F32, tag="obs")
                nc.sync.dma_start(
                    obs_t[:].rearrange("p (j o) -> p j o", o=OBS), obs_v[c]
                )
                # paired transposes: [128,128] -> [128,128]; pair h stacks
                # blocks (2h | 2h+1) on the low/high partition halves
                pT = pTp.tile([128, 512], F32, tag="pT")
                for h in range(4):
                    nc.tensor.transpose(
                        pT[:, h * 128:(h + 1) * 128],
                        obs_t[:, h * 128:(h + 1) * 128],
                        ident[:],
                    )
                obsT = wp.tile([128, 512], F32, tag="obsT")
                nc.scalar.copy(obsT[:], pT[:])

                # graph sums of obs (block j -> permuted col slot)
                sobs_p = psmp.tile([OBS, GPC], F32, tag="sm")
                for j in range(8):
                    pos = BLOCK_POS[j]
                    nc.tensor.matmul(
                        sobs_p[:, pos * 8:(pos + 1) * 8],
                        obs_t[:, j * OBS:(j + 1) * OBS],
                        p8[:],
                    )
                sobs = wp.tile([OBS, GPC], F32, tag="sobsb")
                nc.scalar.copy(sobs[:], sobs_p[:])

                # h1 = W01.T @ sobs  (layer-1 aggregate, bias folded in tanh)
                h1_p = psmp.tile([HID, GPC], F32, tag="sm")
                nc.tensor.matmul(h1_p[:], w01[:], sobs[:])
                h1 = wp.tile([HID, GPC], F32, tag="h1")
                nc.scalar.copy(h1[:], h1_p[:])

                # x0 via zero-padded weights: cols 0:512 <- low partition
                # rows (even blocks), cols 512:1024 <- high rows (odd blocks)
                x0_p = px0p.tile([HID, CHUNK], F32, tag="x0")
                nc.tensor.matmul(x0_p[:, 0:512], wembA[:], obsT[:])
                nc.tensor.matmul(x0_p[:, 512:1024], wembB[:], obsT[:])
                stash[c] = (x0_p, h1)

            def stage_b1(c):
                x0_p, h1 = stash.pop(c)
                u1 = wp.tile([HID, CHUNK], F32, tag="u1")
                nc.vector.tensor_add(
                    u1[:].rearrange("h (g s) -> h g s", s=N_AGENTS),
                    x0_p[:].rearrange("h (g s) -> h g s", s=N_AGENTS),
                    h1[:].rearrange("h (g o) -> h g o", o=1).broadcast_to(
                        [HID, GPC, N_AGENTS]
                    ),
                )
                x1 = wp.tile([HID, CHUNK], F32, tag="x1")
                nc.scalar.activation(x1[:], u1[:], AF.Tanh, bias=b1f[:])

                sx1 = wp.tile([HID, GPC], F32, tag="sx1")
                nc.vector.reduce_sum(
                    sx1[:],
                    x1[:].rearrange("h (g s) -> h g s", s=N_AGENTS),
                    axis=AX.X,
                )
                h2_p = psmp.tile([HID, GPC], F32, tag="sm")
                nc.tensor.matmul(h2_p[:], w2[:], sx1[:])
                h2 = wp.tile([HID, GPC], F32, tag="h2")
                nc.scalar.copy(h2[:], h2_p[:])
                stash2[c] = (x1, h2)

            def stage_b2a(c):
                x1, h2 = stash2.pop(c)
                u2 = wp.tile([HID, CHUNK], F32, tag="u2")
                nc.gpsimd.tensor_add(
                    u2[:].rearrange("h (g s) -> h g s", s=N_AGENTS),
                    x1[:].rearrange("h (g s) -> h g s", s=N_AGENTS),
                    h2[:].rearrange("h (g o) -> h g o", o=1).broadcast_to(
                        [HID, GPC, N_AGENTS]
                    ),
                )
                x2 = wp.tile([HID, CHUNK], F32, tag="x2")
                nc.scalar.activation(x2[:], u2[:], AF.Tanh, bias=b2[:])
                stash3[c] = x2

            def stage_b2b(c):
                x2 = stash3.pop(c)
                sx2 = wp.tile([HID, GPC], F32, tag="sx2")
                nc.vector.reduce_sum(
                    sx2[:],
                    x2[:].rearrange("h (g s) -> h g s", s=N_AGENTS),
                    axis=AX.X,
                )
                y_p = psmp.tile([1, GPC], F32, tag="sm")
                nc.tensor.matmul(y_p[:], wfc[:], sx2[:])
                nc.vector.tensor_copy(outsb[0:1, c * GPC:(c + 1) * GPC], y_p[:])

            stash2 = {}
            stash3 = {}
            stage_a(0)
            stage_a(1)
            stage_b1(0)
            stage_a(2)
            stage_b1(1)
            stage_b2a(0)
            for c in range(NCHUNK):
                if c + 3 < NCHUNK:
                    stage_a(c + 3)
                if c + 2 < NCHUNK:
                    stage_b1(c + 2)
                if c + 1 < NCHUNK:
                    stage_b2a(c + 1)
                stage_b2b(c)

            nc.sync.dma_start(out_d[:], outsb[:])

    nc.compile()
    return nc


def _get_nc():
    if "nc" not in _CACHE:
        _CACHE["nc"] = _build_nc()
    return _CACHE["nc"]


def _make_in_maps(cent_obs, w_emb, b_emb, w_gcn, b_gcn, w_fc1):
    w_emb = np.ascontiguousarray(w_emb, np.float32)
    wembz = np.zeros((2, 128, HID), np.float32)
    wembz[0, :OBS] = w_emb
    wembz[1, OBS:] = w_emb
    w01 = np.ascontiguousarray(w_emb @ (w_gcn[0] / np.float32(16.0)), np.float32)
    w2s = np.ascontiguousarray(w_gcn[1] / np.float32(16.0), np.float32)
    # tanh1 bias: b_gcn[0] + b_emb (residual path) + b_emb @ w_gcn[0] (agg path)
    b1f = (b_gcn[0] + b_emb + b_emb @ w_gcn[0]).astype(np.float32).reshape(HID, 1)
    b2 = b_gcn[1].astype(np.float32).reshape(HID, 1)
    wfc = w_fc1.astype(np.float32).reshape(HID, 1)
    p8 = np.zeros((128, 8), np.float32)
    p8[np.arange(128), np.arange(128) // N_AGENTS] = 1.0
    ident = np.eye(128, dtype=np.float32)
    shared = {
        "wemb": wembz, "w01": w01, "w2s": w2s, "b1f": b1f, "b2": b2,
        "wfc": wfc, "p8": p8, "ident": ident,
    }
    in_maps = []
    for ci in range(N_CORES):
        m = dict(shared)
        m["obs"] = np.ascontiguousarray(
            cent_obs[ci * NPC:(ci + 1) * NPC], np.float32
        )
        in_maps.append(m)
    return in_maps


def kernel(cent_obs, w_emb, b_emb, w_gcn, b_gcn, w_fc1, b_fc1,
           edge_src, edge_dst, _trace=False):
    cent_obs = np.asarray(cent_obs, np.float32)
    nc = _get_nc()
    in_maps = _make_in_maps(
        cent_obs, np.asarray(w_emb, np.float32), np.asarray(b_emb, np.float32),
        np.asarray(w_gcn, np.float32), np.asarray(b_gcn, np.float32),
        np.asarray(w_fc1, np.float32),
    )
    kw = {}
    if _trace:
        kw = dict(trace=True)
    res = run_bass_kernel_spmd(nc, in_maps, list(range(N_CORES)), **kw)
    y = np.concatenate(
        [np.asarray(res.results[i]["out"]).reshape(-1) for i in range(N_CORES)]
    )
    # undo per-chunk block permutation [b0, b2, b1, b3]
    y = y.reshape(-1, 8, 8)[:, np.argsort(BLOCK_PERM), :].reshape(-1)
    out = (y / np.float32(16.0) + np.float32(np.asarray(b_fc1).reshape(()))).astype(
        np.float32
    )
    if _trace:
        _CACHE["last_result"] = res
    return out.reshape(BATCH, 1)

